# revision 4
# baseline (speedup 1.0000x reference)
"""Trainium2 Bass kernel for nn_Depthwise: binarized depthwise 3x3 conv forward.

    out = dwconv(sign(x), w) + dwconv(x, sign(w)),  stride 1, pad 1
    x: [32, 128, 112, 112] f32, w: [128, 1, 3, 3] f32, alphas: scalars (forward
    value of the STE sign is just sign(); alphas only shape gradients).

Strategy (8 NeuronCores, channel-sharded):
  - Each core gets 16 channels x all 32 images (25.7 MB in, 25.7 MB out).
  - Depthwise 3x3 conv runs on the TensorEngine as banded matmuls: for each
    channel and each kernel column b, a [112,112] tridiagonal-band lhsT
    (built on host from the 3 weights of column b) contracts over H, and the
    W-shift (b-1) is applied by accumulating into PSUM at a +-1 column offset.
    6 weight passes per channel (2 convs x 3 kernel columns) accumulate the
    full output in PSUM.
  - sign(x) on ScalarE (ACT Sign), f32->bf16 cast on VectorE; matmuls in bf16
    (products are exact; only the bf16 rounding of x and w contributes error,
    measured rel err ~1.7e-3 vs the f32 reference).
  - Images are packed per channel into one [112, 3624] slab with zero
    separator columns so each channel is ONE input DMA and ONE output DMA
    (the conv's zero padding comes from the separators + band-edge clipping).
  - Weight-pass-outer / image-group-inner loop order lets one LDWEIGHTS serve
    8 matmuls (redundant LDWs are deduped post-Tile-scheduling).
"""

import contextlib

import numpy as np
import ml_dtypes

import concourse.bacc as bacc
import concourse.mybir as mybir
from concourse.tile import TileContext
from concourse.bass_utils import run_bass_kernel_spmd

F32 = mybir.dt.float32
BF16 = mybir.dt.bfloat16

N_CORES = 8
N_IMG = 32
C_TOTAL = 128
NCH = C_TOTAL // N_CORES   # 16 channels per core
IPG = 4                    # images per group (PSUM free-dim limit: 453 <= 512)
NG = N_IMG // IPG          # 8 groups
H = 112
W = 112
WP = IPG * (W + 1) + 1     # 453: [z img0 z img1 z img2 z img3 z]
WB = NG * WP               # 3624


def _dedup_ldweights(nc):
    """Remove consecutive InstLdweights with identical weight APs within a
    basic block; merge their waits into the next PE instruction. Safe because
    PE stationary weights persist until the next Ldweights."""
    import concourse.mybir as mb

    def walk(bb):
        cur = None
        drop = []
        for ins in bb.instructions:
            t = type(ins).__name__
            if t == "InstLdweights":
                k = str(ins.ins[0])
                if cur is not None and k == cur:
                    drop.append(ins)
                else:
                    cur = k
            elif hasattr(ins, "blocks"):
                for b2 in ins.blocks:
                    walk(b2)
                cur = None
        if not drop:
            return
        drop_ids = {id(i) for i in drop}
        pending = []
        for ins in list(bb.instructions):
            if id(ins) in drop_ids:
                si = ins.sync_info
                if si is not None:
                    assert not si.on_update
                    pending.extend(si.on_wait)
                continue
            if pending and ins.engine == mb.EngineType.PE:
                si = ins.sync_info
                if si is None:
                    ins.sync_info = mb.SyncInfo(on_wait=list(pending), on_update=[])
                else:
                    si.on_wait = list(si.on_wait) + pending
                    ins.sync_info = si
                pending = []
        assert not pending
        for ins in drop:
            bb.instructions.remove(ins)

    for f in nc.m.functions:
        for bb in f.blocks:
            walk(bb)


def build_nc():
    nc = bacc.Bacc(trn_type="TRN2")
    xp = nc.dram_tensor("xp", [NCH, H, WB], F32, kind="ExternalInput")
    bands = nc.dram_tensor("bands", [H, NCH * 6 * H], BF16, kind="ExternalInput")
    out = nc.dram_tensor("out", [NCH, H, WB], F32, kind="ExternalOutput")

    with TileContext(nc) as tc:
        with (
            tc.tile_pool(name="bandp", bufs=1) as bandp,
            tc.tile_pool(name="xin", bufs=3) as xinp,
            tc.tile_pool(name="xbf", bufs=3) as xbfp,
            tc.tile_pool(name="sbf", bufs=3) as sbfp,
            tc.tile_pool(name="ps", bufs=1, space="PSUM") as psp,
            tc.tile_pool(name="ot", bufs=3) as otp,
        ):
            band_tiles = {}

            def Wmat(c, j):
                return band_tiles[c][:, j * H:(j + 1) * H]

            for c in range(NCH):
                xt = xinp.tile([H, WB], F32)
                nc.sync.dma_start(xt[:, :], xp[c, :, :])
                # per-channel band load right behind the channel's input DMA
                bc = bandp.tile([H, 6 * H], BF16, name=f"bands{c}", tag=f"bands{c}")
                nc.sync.dma_start(bc[:, :], bands[:, c * 6 * H:(c + 1) * 6 * H])
                band_tiles[c] = bc
                xb = xbfp.tile([H, WB], BF16)
                nc.vector.tensor_copy(xb[:, :], xt[:, :])
                st = sbfp.tile([H, WB], BF16)
                nc.scalar.activation(st[:, :], xt[:, :],
                                     mybir.ActivationFunctionType.Sign)
                # re-zero separator cols in case Sign(0) != 0 on HW
                nc.vector.memset(
                    st[:, 0:WB].rearrange("p (g q) -> p g q", q=WP)[:, :, 0:WP:(W + 1)],
                    0.0)
                ot = otp.tile([H, WB], F32)
                pss = [psp.tile([H, WP], F32, name=f"ps{g}", tag=f"ps{g}")
                       for g in range(NG)]
                # weight-pass outer, group inner: one LDW per 8 matmuls
                for ri, rhs in enumerate((xb, st)):
                    for bi in range(3):
                        wm = Wmat(c, ri * 3 + bi)
                        first = ri == 0 and bi == 0
                        last = ri == 1 and bi == 2
                        for g in range(NG):
                            j0 = g * WP
                            ps = pss[g]
                            if bi == 0:    # b=1 (dz=0)
                                nc.tensor.matmul(ps[:, 0:WP], wm,
                                                 rhs[:, j0:j0 + WP],
                                                 start=first, stop=False)
                            elif bi == 1:  # b=0 (dz=-1): out[1:], in[:-1]
                                nc.tensor.matmul(ps[:, 1:WP], wm,
                                                 rhs[:, j0:j0 + WP - 1],
                                                 start=False, stop=False)
                            else:          # b=2 (dz=+1): out[:-1], in[1:]
                                nc.tensor.matmul(ps[:, 0:WP - 1], wm,
                                                 rhs[:, j0 + 1:j0 + WP],
                                                 start=False, stop=last)
                for g in range(NG):
                    j0 = g * WP
                    if g % 2 == 0:
                        nc.scalar.copy(ot[:, j0:j0 + WP], pss[g][:, :])
                    else:
                        nc.vector.tensor_copy(ot[:, j0:j0 + WP], pss[g][:, :])
                nc.gpsimd.dma_start(out[c, :, :], ot[:, :])

    _dedup_ldweights(nc)
    nc.finalize()
    return nc


def make_bands(weight):
    """weight: [NCH, 3, 3] f32 -> bands bf16 [H, NCH*6*H] (k-major).

    lhsT_b[k, m] = w[a = k-m+1, b]; passes ordered [x/b1, x/b0, x/b2,
    s/b1, s/b0, s/b2] (x-conv first: it only needs the DVE cast, not the
    slower ACT sign, shortening the per-channel critical path)."""
    B = np.zeros((NCH, 6, H, H), np.float32)
    sgn = np.sign(weight)
    for c in range(NCH):
        for bi, b in enumerate((1, 0, 2)):
            for a in range(3):
                eye = np.eye(H, k=1 - a, dtype=np.float32)
                B[c, 3 + bi] += weight[c, a, b] * eye
                B[c, bi] += sgn[c, a, b] * eye
    Bt = B.transpose(2, 0, 1, 3).reshape(H, NCH * 6 * H)
    return np.ascontiguousarray(Bt.astype(ml_dtypes.bfloat16))


def pack_x(xc):
    """xc: [NCH, N_IMG, H, W] f32 -> [NCH, H, WB] f32 packed + zero separators."""
    buf = np.zeros((NCH, NG, IPG, H, W + 1), np.float32)
    buf[..., 1:] = xc.reshape(NCH, NG, IPG, H, W)
    t = buf.transpose(0, 3, 1, 2, 4).reshape(NCH, H, NG, IPG * (W + 1))
    res = np.zeros((NCH, H, NG, WP), np.float32)
    res[..., :IPG * (W + 1)] = t
    return res.reshape(NCH, H, WB)


def unpack_out(o):
    """o: [NCH, H, WB] -> [NCH, N_IMG, H, W], image n = g*IPG + i."""
    og = o.reshape(NCH, H, NG, WP)[..., :IPG * (W + 1)]
    og = og.reshape(NCH, H, NG, IPG, W + 1)[..., 1:]
    return og.transpose(0, 2, 3, 1, 4).reshape(NCH, N_IMG, H, W)


def kernel(x, weight, alpha_x=None, alpha_w=None):
    """Full inputs in, full output out. Shards channels across 8 cores."""
    x = np.ascontiguousarray(np.asarray(x, dtype=np.float32))
    weight = np.asarray(weight, dtype=np.float32).reshape(C_TOTAL, 3, 3)

    X = x.transpose(1, 0, 2, 3)  # [C, N, H, W]
    in_maps = []
    for k in range(N_CORES):
        cs = slice(NCH * k, NCH * (k + 1))
        in_maps.append({
            "xp": pack_x(X[cs]),
            "bands": make_bands(weight[cs]),
        })

    nc = build_nc()
    res = run_bass_kernel_spmd(nc, in_maps, core_ids=list(range(N_CORES)))

    got = np.empty((N_IMG, C_TOTAL, H, W), np.float32)
    for k in range(N_CORES):
        o = unpack_out(res.results[k]["out"])  # [NCH, N_IMG, H, W]
        got[:, NCH * k:NCH * (k + 1)] = o.transpose(1, 0, 2, 3)
    return got
